# revision 30
# baseline (speedup 1.0000x reference)
"""2-layer GRU (B=64, T=256, D=64, H=1024) + final linear on TRN2, 8 cores.

Strategy: data-parallel over batch (8 rows per core, no collectives).
v2: layer-1's input projection (h1 @ Wi1) is hoisted out of the recurrence
into per-chunk GEMMs with M=64 (8 timesteps x 8 batch packed as stationary
columns -> full PE-array utilization), pipelined 2 chunks behind layer 0:
  slot s:  L0-recurrence(chunk s) || Wi1-GEMM(chunk s-1) || L1-rec(chunk s-2)
The GEMM matmuls are emitted between recurrence matmul groups so they fill
the PE while the sigmoid/tanh/update tails run on ACT/DVE/Pool.
Recurrence matmuls run as four concurrent col-tiled PE streams
(tile_position (0,32g)); gate order per strip is r,z,n with r first so the
r-dependent part of the tail starts as early as possible.
"""
import numpy as np
import ml_dtypes

import concourse.bass as bass
import concourse.tile as tile
from concourse import bacc, mybir

F32 = mybir.dt.float32
BF16 = mybir.dt.bfloat16
AF = mybir.ActivationFunctionType
ALU = mybir.AluOpType

B = 8            # batch rows per core
H = 1024
KC = H // 128    # 8 K-chunks
T = 256
CH = 4           # timesteps per chunk
NCH = T // CH    # 32 chunks
N_CORES = 8

_cache = {}


def _build(unroll=8, repeat=1):
    nc = bacc.Bacc("TRN2", target_bir_lowering=False, debug=False,
                   enable_asserts=False, num_devices=N_CORES)

    xT_d = nc.dram_tensor("xT", [128, T + 2 * CH, B], BF16, kind="ExternalInput")
    Wx0_d = nc.dram_tensor("Wx0", [128, 4, 3, 256], BF16, kind="ExternalInput")
    Wh0_d = nc.dram_tensor("Wh0", [128, KC, 4, 3, 256], BF16, kind="ExternalInput")
    Wh1_d = nc.dram_tensor("Wh1", [128, KC, 4, 3, 256], BF16, kind="ExternalInput")
    Wi1_d = nc.dram_tensor("Wi1", [128, KC, 4, 768], BF16, kind="ExternalInput")
    bi1_d = nc.dram_tensor("bi1", [128, 4, 768], BF16, kind="ExternalInput")
    bhn0_d = nc.dram_tensor("bhn0", [128, 4, 256], BF16, kind="ExternalInput")
    bhn1_d = nc.dram_tensor("bhn1", [128, 4, 256], BF16, kind="ExternalInput")
    id_d = nc.dram_tensor("ident", [128, 128], BF16, kind="ExternalInput")
    fcw_d = nc.dram_tensor("fcw", [128, KC, 1], BF16, kind="ExternalInput")
    out_d = nc.dram_tensor("out", [B, 1], F32, kind="ExternalOutput")

    KORD = [0, 2, 4, 6, 1, 3, 5, 7]
    with tile.TileContext(nc) as tc:
        with (
            tc.tile_pool(name="weights", bufs=1) as wpool,
            tc.tile_pool(name="state", bufs=1) as spool,
            tc.tile_pool(name="work", bufs=1) as work,
            tc.tile_pool(name="xstp", bufs=1) as xstp,
            tc.tile_pool(name="pgates", bufs=1, space="PSUM") as pg,
            tc.tile_pool(name="ptrans", bufs=1, space="PSUM") as ptp,
            tc.tile_pool(name="pgemm", bufs=2, space="PSUM") as pgg_pool,
        ):
            # ---- weight/const loads (L0's deps first so compute starts early)
            xTs = wpool.tile([128, T + 2 * CH, B], BF16, tag="xTs")
            nc.sync.dma_start(xTs[:], xT_d.ap())
            Wx0s = wpool.tile([128, 4, 3, 256], BF16, tag="Wx0s")
            nc.sync.dma_start(Wx0s[:], Wx0_d.ap())
            Wh0s = wpool.tile([128, KC, 4, 3, 256], BF16, tag="Wh0s")
            nc.sync.dma_start(Wh0s[:], Wh0_d.ap())
            bhn0s = wpool.tile([128, 4, 256], BF16, tag="bhn0s")
            nc.sync.dma_start(bhn0s[:], bhn0_d.ap())
            idf = wpool.tile([128, 128], BF16, tag="idf")
            nc.sync.dma_start(idf[:], id_d.ap())
            Wh1s = wpool.tile([128, KC, 4, 3, 256], BF16, tag="Wh1s")
            nc.sync.dma_start(Wh1s[:], Wh1_d.ap())
            Wi1s = wpool.tile([128, KC, 4, 768], BF16, tag="Wi1s")
            nc.sync.dma_start(Wi1s[:], Wi1_d.ap())
            bi1s = wpool.tile([128, 4, 768], BF16, tag="bi1s")
            nc.sync.dma_start(bi1s[:], bi1_d.ap())
            bhn1s = wpool.tile([128, 4, 256], BF16, tag="bhn1s")
            nc.sync.dma_start(bhn1s[:], bhn1_d.ap())
            fcws = wpool.tile([128, KC, 1], BF16, tag="fcws")
            nc.sync.dma_start(fcws[:], fcw_d.ap())
            onesb = wpool.tile([128, B], BF16, tag="onesb")
            nc.gpsimd.memset(onesb[:], 0.0)
            nc.gpsimd.memset(onesb[0:1, :], 1.0)
            onesg = wpool.tile([128, 32], BF16, tag="onesg")
            nc.gpsimd.memset(onesg[:], 0.0)
            nc.gpsimd.memset(onesg[0:1, :], 1.0)

            # ---- state
            h0s = spool.tile([128, 256], BF16, tag="h0s")
            h2s = spool.tile([128, 256], BF16, tag="h2s")
            hT2 = spool.tile([128, 2, 128], BF16, tag="hT2")
            hT0c = [spool.tile([128, 2, 128, CH], BF16, tag=f"hT0c{p}", name=f"hT0c{p}")
                    for p in range(2)]
            xg1c = [spool.tile([128, CH, 3, 256], BF16, tag=f"xg1c{p}", name=f"xg1c{p}")
                    for p in range(2)]
            tmp = [spool.tile([128, 768], BF16, tag=f"tmp{p}", name=f"tmp{p}")
                   for p in range(2)]
            nc.gpsimd.memset(h0s[:], 0.0)
            nc.gpsimd.memset(h2s[:], 0.0)
            nc.gpsimd.memset(hT2[:], 0.0)
            nc.gpsimd.memset(hT0c[0][:], 0.0)
            nc.gpsimd.memset(hT0c[1][:], 0.0)

            def hT2_chunk(k):
                return hT2[:, k % 2, 32 * (k // 2):32 * (k // 2) + B]

            def hT0_stat(par, tau):
                """Stationary slices fn for L0 step tau of chunk parity par."""
                if tau == 0:
                    buf, t = hT0c[1 - par], CH - 1
                else:
                    buf, t = hT0c[par], tau - 1
                return lambda k: buf[:, k % 2, 32 * (k // 2):32 * (k // 2) + B, t]

            # ---------------- layer 0 step ----------------
            # PSUM is bank-granular (2KB/partition per tag): pack two
            # [128,256] gate psums per bank.
            def l0_mms(stat, xst):
                prx = pg.tile([128, 512], F32, tag="prx0")
                pnz = pg.tile([128, 512], F32, tag="pnz0")
                pr = prx[:, 0:256]
                px = prx[:, 256:512]
                pn = pnz[:, 0:256]
                pz = pnz[:, 256:512]
                for g in range(4):
                    nc.tensor.matmul(pr[32 * g:32 * g + B, :], xst,
                                     Wx0s[:, g, 0, :], start=True, stop=False,
                                     tile_position=(0, 32 * g))
                for ki, k in enumerate(KORD):
                    for g in range(4):
                        nc.tensor.matmul(pr[32 * g:32 * g + B, :], stat(k),
                                         Wh0s[:, k, g, 0, :], start=False,
                                         stop=(ki == KC - 1),
                                         tile_position=(0, 32 * g))
                for g in range(4):
                    nc.tensor.matmul(pn[32 * g:32 * g + B, :], onesb[:],
                                     bhn0s[:, g, :], start=True, stop=False,
                                     tile_position=(0, 32 * g))
                for ki, k in enumerate(KORD):
                    for g in range(4):
                        nc.tensor.matmul(pn[32 * g:32 * g + B, :], stat(k),
                                         Wh0s[:, k, g, 2, :], start=False,
                                         stop=(ki == KC - 1),
                                         tile_position=(0, 32 * g))
                for g in range(4):
                    nc.tensor.matmul(px[32 * g:32 * g + B, :], xst,
                                     Wx0s[:, g, 2, :], start=True, stop=True,
                                     tile_position=(0, 32 * g))
                for g in range(4):
                    nc.tensor.matmul(pz[32 * g:32 * g + B, :], xst,
                                     Wx0s[:, g, 1, :], start=True, stop=False,
                                     tile_position=(0, 32 * g))
                for ki, k in enumerate(KORD):
                    for g in range(4):
                        nc.tensor.matmul(pz[32 * g:32 * g + B, :], stat(k),
                                         Wh0s[:, k, g, 1, :], start=False,
                                         stop=(ki == KC - 1),
                                         tile_position=(0, 32 * g))
                return pr, pn, px, pz

            def l0_tail(pr, pn, px, pz):
                r_t = work.tile([128, 256], BF16, tag="r_t0")
                nc.scalar.activation(r_t[:], pr[:], AF.Sigmoid)
                t1 = work.tile([128, 256], BF16, tag="t10")
                nc.vector.scalar_tensor_tensor(t1[:], pn[:], 1.0, r_t[:],
                                               op0=ALU.mult, op1=ALU.mult)
                pre_n = work.tile([128, 256], BF16, tag="pre_n0")
                nc.vector.scalar_tensor_tensor(pre_n[:], t1[:], 0.0, px[:],
                                               op0=ALU.add, op1=ALU.add)
                n_t = work.tile([128, 256], BF16, tag="n_t0")
                nc.scalar.activation(n_t[:], pre_n[:], AF.Tanh)
                z_t = work.tile([128, 256], BF16, tag="z_t0")
                nc.scalar.activation(z_t[:], pz[:], AF.Sigmoid)
                d = work.tile([128, 256], BF16, tag="d0")
                nc.vector.tensor_sub(d[:], h0s[:], n_t[:])
                t2 = work.tile([128, 256], BF16, tag="t20")
                nc.vector.tensor_mul(t2[:], d[:], z_t[:])
                nc.vector.tensor_add(h0s[:], t2[:], n_t[:])

            def l0_tr(par, tau):
                pt = ptp.tile([128, 512], BF16, tag="ptx", name="ptx")[:, 0:256]
                for half in range(2):
                    nc.tensor.transpose(pt[:, 128 * half:128 * half + 128],
                                        h0s[:, 128 * half:128 * half + 128],
                                        idf[:])
                return pt

            # ---------------- layer 1 step ----------------
            def l1_r_mms(pr):
                for ki, k in enumerate(KORD):
                    for g in range(4):
                        nc.tensor.matmul(pr[32 * g:32 * g + B, :], hT2_chunk(k),
                                         Wh1s[:, k, g, 0, :], start=(ki == 0),
                                         stop=(ki == KC - 1),
                                         tile_position=(0, 32 * g))

            def l1_n_mms(pn):
                for g in range(4):
                    nc.tensor.matmul(pn[32 * g:32 * g + B, :], onesb[:],
                                     bhn1s[:, g, :], start=True, stop=False,
                                     tile_position=(0, 32 * g))
                for ki, k in enumerate(KORD):
                    for g in range(4):
                        nc.tensor.matmul(pn[32 * g:32 * g + B, :], hT2_chunk(k),
                                         Wh1s[:, k, g, 2, :], start=False,
                                         stop=(ki == KC - 1),
                                         tile_position=(0, 32 * g))

            def l1_z_mms(pz):
                for ki, k in enumerate(KORD):
                    for g in range(4):
                        nc.tensor.matmul(pz[32 * g:32 * g + B, :], hT2_chunk(k),
                                         Wh1s[:, k, g, 1, :], start=(ki == 0),
                                         stop=(ki == KC - 1),
                                         tile_position=(0, 32 * g))

            def l1_tail_a(pr, pn, xgpar, tau):
                xg = xg1c[xgpar]
                rpre = work.tile([128, 256], F32, tag="rpre1")
                nc.vector.scalar_tensor_tensor(rpre[:], pr[:], 0.0,
                                               xg[:, tau, 0, :],
                                               op0=ALU.add, op1=ALU.add)
                r_t = work.tile([128, 256], BF16, tag="r_t1")
                nc.scalar.activation(r_t[:], rpre[:], AF.Sigmoid)
                t1 = work.tile([128, 256], BF16, tag="t11")
                nc.vector.scalar_tensor_tensor(t1[:], pn[:], 1.0, r_t[:],
                                               op0=ALU.mult, op1=ALU.mult)
                pre_n = work.tile([128, 256], BF16, tag="pre_n1")
                nc.vector.scalar_tensor_tensor(pre_n[:], t1[:], 0.0,
                                               xg[:, tau, 2, :],
                                               op0=ALU.add, op1=ALU.add)
                n_t = work.tile([128, 256], BF16, tag="n_t1")
                nc.scalar.activation(n_t[:], pre_n[:], AF.Tanh)
                return n_t

            def l1_tail_b(pz, n_t, xgpar, tau):
                xg = xg1c[xgpar]
                zpre = work.tile([128, 256], F32, tag="zpre1")
                nc.vector.scalar_tensor_tensor(zpre[:], pz[:], 0.0,
                                               xg[:, tau, 1, :],
                                               op0=ALU.add, op1=ALU.add)
                z_t = work.tile([128, 256], BF16, tag="z_t1")
                nc.scalar.activation(z_t[:], zpre[:], AF.Sigmoid)
                d = work.tile([128, 256], BF16, tag="d1")
                nc.vector.tensor_sub(d[:], h2s[:], n_t[:])
                t2 = work.tile([128, 256], BF16, tag="t21")
                nc.vector.tensor_mul(t2[:], d[:], z_t[:])
                nc.vector.tensor_add(h2s[:], t2[:], n_t[:])

            def l1_tr():
                pt = ptp.tile([128, 512], BF16, tag="ptx", name="ptx")[:, 256:512]
                for half in range(2):
                    nc.tensor.transpose(pt[:, 128 * half:128 * half + 128],
                                        h2s[:, 128 * half:128 * half + 128],
                                        idf[:])
                return pt

            pend = [False]

            def flush_l1_tr():
                # Deferred L1 transpose+copy: emitted at the head of the NEXT
                # step-pair, when h2s is long since written, so the PE never
                # waits on the sigma-z/t2/h' chain.
                if not pend[0]:
                    return
                pend[0] = False
                pt1 = l1_tr()
                nc.scalar.activation(hT2[:, 0, :], pt1[:, 0:128], AF.Copy)
                nc.scalar.activation(hT2[:, 1, :], pt1[:, 128:256], AF.Copy)

            # ---------------- Wi1 GEMM for one chunk ----------------
            # M=32 stationary (8 batch x 4 steps) per col-strip, 4 strips
            # each streaming one g-block's 768 gate-columns (2 N-passes).
            PASSES = ((0, 512), (512, 768))

            def gemm_blocks(src_par, dst_par):
                buf = hT0c[src_par]

                def statg(k):
                    return buf[:, k % 2, 32 * (k // 2):32 * (k // 2) + B, 0:CH]

                emits = []
                for (c0, c1) in PASSES:
                    state = {}

                    def mk_bias(c0=c0, c1=c1, state=state):
                        def emit():
                            state["pgt"] = pgg_pool.tile([128, 512], F32,
                                                         tag="pgg", name="pgg")
                            for a in range(4):
                                nc.tensor.matmul(
                                    state["pgt"][32 * a:32 * a + 32, 0:c1 - c0],
                                    onesg[:], bi1s[:, a, c0:c1],
                                    start=True, stop=False,
                                    tile_position=(0, 32 * a))
                        return emit

                    def mk_k(k, c0=c0, c1=c1, state=state):
                        def emit():
                            for a in range(4):
                                nc.tensor.matmul(
                                    state["pgt"][32 * a:32 * a + 32, 0:c1 - c0],
                                    statg(k), Wi1s[:, k, a, c0:c1],
                                    start=False, stop=(k == KC - 1),
                                    tile_position=(0, 32 * a))
                        return emit

                    def mk_evac(c0=c0, c1=c1, state=state):
                        def emit():
                            nc.scalar.activation(tmp[dst_par][:, c0:c1],
                                                 state["pgt"][:, 0:c1 - c0],
                                                 AF.Copy)
                        return emit

                    emits.append(mk_bias())
                    for k in range(KC):
                        emits.append(mk_k(k))
                    emits.append(mk_evac())

                def dmas():
                    for g in range(4):
                        nc.sync.dma_start(
                            xg1c[dst_par][32 * g:32 * g + B, :, :, :],
                            tmp[dst_par][32 * g:32 * g + 32, :])
                emits.append(dmas)
                return emits

            def take(it, n):
                for _ in range(n):
                    b = next(it, None)
                    if b is not None:
                        b()

            # ---------------- slot emitters ----------------
            def emit_slot(l0_chunk_par, xst_of, gemm_emits, l1_on,
                          prefetch_of=None):
                """One slot: 8 interleaved L0/L1 steps + GEMM fillers.
                l0_chunk_par: parity of the L0 chunk being produced (or None).
                xst_of(u): stationary AP for L0 step u.
                gemm_emits: iterator of GEMM emit closures (or None).
                l1_on: (xg_parity, ) or None."""
                git = iter(gemm_emits) if gemm_emits is not None else iter(())
                for u in range(CH):
                    flush_l1_tr()
                    if l0_chunk_par is not None:
                        stat = hT0_stat(l0_chunk_par, u)
                        prpnpxpz = l0_mms(stat, xst_of(u))
                        l0_tail(*prpnpxpz)
                    if prefetch_of is not None:
                        prefetch_of(u)
                    pt0 = None
                    if l1_on is not None:
                        prz = pg.tile([128, 512], F32, tag="prz1", name="prz1")
                        pn1 = pg.tile([128, 256], F32, tag="pn1", name="pn1")
                        l1_r_mms(prz[:, 0:256])
                        l1_n_mms(pn1[:])
                        n_t1 = l1_tail_a(prz[:, 0:256], pn1[:], l1_on, u)
                        l1_z_mms(prz[:, 256:512])
                        l1_tail_b(prz[:, 256:512], n_t1, l1_on, u)
                    if l0_chunk_par is not None:
                        pt0 = l0_tr(l0_chunk_par, u)
                    take(git, 6)
                    if pt0 is not None:
                        nc.scalar.activation(hT0c[l0_chunk_par][:, 0, :, u],
                                             pt0[:, 0:128], AF.Copy)
                        nc.scalar.activation(hT0c[l0_chunk_par][:, 1, :, u],
                                             pt0[:, 128:256], AF.Copy)
                    if l1_on is not None:
                        pend[0] = True
                # drain remaining gemm blocks
                take(git, 40)

            # ---------------- prologue: chunks 0 and 1 ----------------
            def xst_static(c):
                return lambda u: xTs[:, CH * c + u, :]

            emit_slot(0, xst_static(0), None, None)

            xstE = [xstp.tile([128, B], BF16, tag=f"xstE{u}", name=f"xstE{u}") for u in range(CH)]
            xstO = [xstp.tile([128, B], BF16, tag=f"xstO{u}", name=f"xstO{u}") for u in range(CH)]

            def pro_prefetch(u):
                # preload xst tiles for main-loop iteration 0
                nc.scalar.activation(xstE[u][:], xTs[:, 2 * CH + u, :], AF.Copy)
                nc.scalar.activation(xstO[u][:], xTs[:, 3 * CH + u, :], AF.Copy)

            emit_slot(1, xst_static(1), gemm_blocks(0, 0), None,
                      prefetch_of=pro_prefetch)

            # ---------------- main loop: 15 iterations x 2 slots ----------
            pend[0] = True   # ensure the loop body traces the head flush
            n_iter = (NCH - 2) // 2 * repeat
            with tc.For_i(0, n_iter, 1) as iv:
                for sl, xst_tiles in ((0, xstE), (1, xstO)):
                    base = 2 * CH + CH * sl

                    def xst_of(u, xst_tiles=xst_tiles):
                        return xst_tiles[u][:]

                    def prefetch(u, xst_tiles=xst_tiles, base=base):
                        if repeat == 1:
                            src = xTs[:, bass.ds(iv * 2 * CH + base + 2 * CH + u, 1), :].opt()
                        else:
                            src = xTs[:, base + u, :]
                        nc.scalar.activation(xst_tiles[u][:], src, AF.Copy)

                    l0_par = sl          # chunk 2i+2+sl has parity sl
                    gem = gemm_blocks(1 - sl, 1 - sl)   # chunk 2i+1+sl
                    l1_par = sl          # chunk 2i+sl
                    emit_slot(l0_par, xst_of, gem, l1_par, prefetch_of=prefetch)

            # ---------------- epilogue ----------------
            # GEMM(chunk 31, parity 1) + L1(chunk 30, parity 0)
            emit_slot(None, None, gemm_blocks(1, 1), 0)
            # L1(chunk 31, parity 1)
            emit_slot(None, None, None, 1)

            flush_l1_tr()

            # ---------------- final projection ----------------
            pfc = ptp.tile([B, 1], F32, tag="pfc")
            for k in range(KC):
                nc.tensor.matmul(pfc[:], hT2_chunk(k), fcws[:, k, :],
                                 start=(k == 0), stop=(k == KC - 1))
            ov = work.tile([B, 1], F32, tag="ov")
            nc.vector.tensor_copy(ov[:], pfc[:])
            nc.sync.dma_start(out_d.ap(), ov[:])

    nc.compile()
    return nc


def _prep_inputs(x, w_ih_l0, w_hh_l0, b_ih_l0, b_hh_l0,
                 w_ih_l1, w_hh_l1, b_ih_l1, b_hh_l1, fc_w, fc_b):
    bf = ml_dtypes.bfloat16
    f32 = np.float32
    x = np.asarray(x, f32)
    w_ih_l0 = np.asarray(w_ih_l0, f32); w_hh_l0 = np.asarray(w_hh_l0, f32)
    b_ih_l0 = np.asarray(b_ih_l0, f32); b_hh_l0 = np.asarray(b_hh_l0, f32)
    w_ih_l1 = np.asarray(w_ih_l1, f32); w_hh_l1 = np.asarray(w_hh_l1, f32)
    b_ih_l1 = np.asarray(b_ih_l1, f32); b_hh_l1 = np.asarray(b_hh_l1, f32)
    fc_w = np.asarray(fc_w, f32)

    def reorder_h(W):
        # W: [3072, 1024] -> [128(p), KC, 4(g), 3(gate), 256(c)]
        Wr = W.reshape(3, 4, 256, KC, 128)      # gate, g, c, k, p
        return np.ascontiguousarray(Wr.transpose(4, 3, 1, 0, 2))

    Wh0 = reorder_h(w_hh_l0).astype(bf)
    Wh1 = reorder_h(w_hh_l1).astype(bf)
    Wi1 = reorder_h(w_ih_l1)                     # [128, KC, 4, 3, 256]
    Wi1 = Wi1.reshape(128, KC, 4, 768).astype(bf)

    # bi1: row0 = input-side biases for l1 in (g, gate, c) flat order.
    bi = np.zeros((128, 4, 768), f32)
    bsum = b_ih_l1 + b_hh_l1
    bvec = np.empty((3, 4, 256), f32)
    bvec[0] = bsum[0:H].reshape(4, 256)
    bvec[1] = bsum[H:2 * H].reshape(4, 256)
    bvec[2] = b_ih_l1[2 * H:3 * H].reshape(4, 256)
    bi[0] = bvec.transpose(1, 0, 2).reshape(4, 768)
    bi1 = bi.astype(bf)

    # Wx0: [128, 4, 3, 256]; rows 0-63 w_ih_l0.T, row 64 biases (r,z: both; n: ih)
    Wx0 = np.zeros((128, 4, 3, 256), f32)
    wi0 = w_ih_l0.reshape(3, 4, 256, 64)         # gate, g, c, d
    Wx0[0:64] = wi0.transpose(3, 1, 0, 2)        # d, g, gate, c
    b0sum = b_ih_l0 + b_hh_l0
    Wx0[64, :, 0, :] = b0sum[0:H].reshape(4, 256)
    Wx0[64, :, 1, :] = b0sum[H:2 * H].reshape(4, 256)
    Wx0[64, :, 2, :] = b_ih_l0[2 * H:3 * H].reshape(4, 256)
    Wx0 = Wx0.astype(bf)

    bhn0 = np.zeros((128, 4, 256), f32)
    bhn0[0] = b_hh_l0[2 * H:3 * H].reshape(4, 256)
    bhn0 = bhn0.astype(bf)
    bhn1 = np.zeros((128, 4, 256), f32)
    bhn1[0] = b_hh_l1[2 * H:3 * H].reshape(4, 256)
    bhn1 = bhn1.astype(bf)

    fcw = np.ascontiguousarray(fc_w.reshape(KC, 128).T).reshape(128, KC, 1)
    fcw = fcw.astype(bf)
    ident = np.eye(128, dtype=f32).astype(bf)

    shared = dict(Wx0=Wx0, Wh0=Wh0, Wh1=Wh1, Wi1=Wi1, bi1=bi1,
                  bhn0=bhn0, bhn1=bhn1, ident=ident, fcw=fcw)
    in_maps = []
    for c in range(N_CORES):
        xs = x[c * B:(c + 1) * B]                 # [B, T, D]
        xTc = np.zeros((128, T + 2 * CH, B), f32)
        xTc[0:64, 0:T, :] = xs.transpose(2, 1, 0)
        xTc[64, 0:T, :] = 1.0
        m = dict(shared)
        m["xT"] = xTc.astype(bf)
        in_maps.append(m)
    return in_maps


def kernel(**inputs) -> np.ndarray:
    from concourse import bass_utils
    if "nc" not in _cache:
        _cache["nc"] = _build()
    nc = _cache["nc"]
    in_maps = _prep_inputs(**inputs)
    res = bass_utils.run_bass_kernel_spmd(nc, in_maps,
                                          core_ids=list(range(N_CORES)))
    out = np.concatenate([res.results[c]["out"] for c in range(N_CORES)], axis=0)
    return (out + np.asarray(inputs["fc_b"], np.float32)).astype(np.float32)


# revision 31
# speedup vs baseline: 1.1066x; 1.1066x over previous
"""2-layer GRU (B=64, T=256, D=64, H=1024) + final linear on TRN2, 8 cores.

Strategy: data-parallel over batch (8 rows per core, no collectives).
v2: layer-1's input projection (h1 @ Wi1) is hoisted out of the recurrence
into per-chunk GEMMs with M=64 (8 timesteps x 8 batch packed as stationary
columns -> full PE-array utilization), pipelined 2 chunks behind layer 0:
  slot s:  L0-recurrence(chunk s) || Wi1-GEMM(chunk s-1) || L1-rec(chunk s-2)
The GEMM matmuls are emitted between recurrence matmul groups so they fill
the PE while the sigmoid/tanh/update tails run on ACT/DVE/Pool.
Recurrence matmuls run as four concurrent col-tiled PE streams
(tile_position (0,32g)); gate order per strip is r,z,n with r first so the
r-dependent part of the tail starts as early as possible.
"""
import numpy as np
import ml_dtypes

import concourse.bass as bass
import concourse.tile as tile
from concourse import bacc, mybir

F32 = mybir.dt.float32
BF16 = mybir.dt.bfloat16
AF = mybir.ActivationFunctionType
ALU = mybir.AluOpType

B = 8            # batch rows per core
H = 1024
KC = H // 128    # 8 K-chunks
T = 256
CH = 4           # timesteps per chunk
NCH = T // CH    # 32 chunks
N_CORES = 8

_cache = {}


def _build(unroll=8, repeat=1):
    nc = bacc.Bacc("TRN2", target_bir_lowering=False, debug=False,
                   enable_asserts=False, num_devices=N_CORES)

    xT_d = nc.dram_tensor("xT", [128, T + 2 * CH, B], BF16, kind="ExternalInput")
    Wx0_d = nc.dram_tensor("Wx0", [128, 4, 3, 256], BF16, kind="ExternalInput")
    Wh0_d = nc.dram_tensor("Wh0", [128, KC, 4, 3, 256], BF16, kind="ExternalInput")
    Wh1_d = nc.dram_tensor("Wh1", [128, KC, 4, 3, 256], BF16, kind="ExternalInput")
    Wi1_d = nc.dram_tensor("Wi1", [128, KC, 4, 768], BF16, kind="ExternalInput")
    bi1_d = nc.dram_tensor("bi1", [128, 4, 768], BF16, kind="ExternalInput")
    bhn0_d = nc.dram_tensor("bhn0", [128, 4, 256], BF16, kind="ExternalInput")
    bhn1_d = nc.dram_tensor("bhn1", [128, 4, 256], BF16, kind="ExternalInput")
    id_d = nc.dram_tensor("ident", [128, 128], BF16, kind="ExternalInput")
    fcw_d = nc.dram_tensor("fcw", [128, KC, 1], BF16, kind="ExternalInput")
    out_d = nc.dram_tensor("out", [B, 1], F32, kind="ExternalOutput")

    KORD = [0, 2, 4, 6, 1, 3, 5, 7]
    with tile.TileContext(nc) as tc:
        with (
            tc.tile_pool(name="weights", bufs=1) as wpool,
            tc.tile_pool(name="state", bufs=1) as spool,
            tc.tile_pool(name="work", bufs=1) as work,
            tc.tile_pool(name="xstp", bufs=1) as xstp,
            tc.tile_pool(name="pgates", bufs=1, space="PSUM") as pg,
            tc.tile_pool(name="ptrans", bufs=1, space="PSUM") as ptp,
            tc.tile_pool(name="pgemm", bufs=2, space="PSUM") as pgg_pool,
        ):
            # ---- weight/const loads (L0's deps first so compute starts early)
            xTs = wpool.tile([128, T + 2 * CH, B], BF16, tag="xTs")
            nc.sync.dma_start(xTs[:], xT_d.ap())
            Wx0s = wpool.tile([128, 4, 3, 256], BF16, tag="Wx0s")
            nc.sync.dma_start(Wx0s[:], Wx0_d.ap())
            Wh0s = wpool.tile([128, KC, 4, 3, 256], BF16, tag="Wh0s")
            nc.sync.dma_start(Wh0s[:], Wh0_d.ap())
            bhn0s = wpool.tile([128, 4, 256], BF16, tag="bhn0s")
            nc.sync.dma_start(bhn0s[:], bhn0_d.ap())
            idf = wpool.tile([128, 128], BF16, tag="idf")
            nc.sync.dma_start(idf[:], id_d.ap())
            Wh1s = wpool.tile([128, KC, 4, 3, 256], BF16, tag="Wh1s")
            nc.sync.dma_start(Wh1s[:], Wh1_d.ap())
            Wi1s = wpool.tile([128, KC, 4, 768], BF16, tag="Wi1s")
            nc.sync.dma_start(Wi1s[:], Wi1_d.ap())
            bi1s = wpool.tile([128, 4, 768], BF16, tag="bi1s")
            nc.sync.dma_start(bi1s[:], bi1_d.ap())
            bhn1s = wpool.tile([128, 4, 256], BF16, tag="bhn1s")
            nc.sync.dma_start(bhn1s[:], bhn1_d.ap())
            fcws = wpool.tile([128, KC, 1], BF16, tag="fcws")
            nc.sync.dma_start(fcws[:], fcw_d.ap())
            onesb = wpool.tile([128, B], BF16, tag="onesb")
            nc.gpsimd.memset(onesb[:], 0.0)
            nc.gpsimd.memset(onesb[0:1, :], 1.0)
            onesg = wpool.tile([128, 32], BF16, tag="onesg")
            nc.gpsimd.memset(onesg[:], 0.0)
            nc.gpsimd.memset(onesg[0:1, :], 1.0)

            # ---- state
            h0s = spool.tile([128, 256], BF16, tag="h0s")
            h2s = spool.tile([128, 256], BF16, tag="h2s")
            hT2 = spool.tile([128, 2, 128], BF16, tag="hT2")
            hT0c = [spool.tile([128, 2, 128, CH], BF16, tag=f"hT0c{p}", name=f"hT0c{p}")
                    for p in range(2)]
            xg1c = [spool.tile([128, CH, 3, 256], BF16, tag=f"xg1c{p}", name=f"xg1c{p}")
                    for p in range(2)]
            tmp = [spool.tile([128, 768], BF16, tag=f"tmp{p}", name=f"tmp{p}")
                   for p in range(2)]
            nc.gpsimd.memset(h0s[:], 0.0)
            nc.gpsimd.memset(h2s[:], 0.0)
            nc.gpsimd.memset(hT2[:], 0.0)
            nc.gpsimd.memset(hT0c[0][:], 0.0)
            nc.gpsimd.memset(hT0c[1][:], 0.0)

            def hT2_chunk(k):
                return hT2[:, k % 2, 32 * (k // 2):32 * (k // 2) + B]

            def hT0_stat(par, tau):
                """Stationary slices fn for L0 step tau of chunk parity par."""
                if tau == 0:
                    buf, t = hT0c[1 - par], CH - 1
                else:
                    buf, t = hT0c[par], tau - 1
                return lambda k: buf[:, k % 2, 32 * (k // 2):32 * (k // 2) + B, t]

            # ---------------- layer 0 step ----------------
            # PSUM is bank-granular (2KB/partition per tag): pack two
            # [128,256] gate psums per bank.
            def l0_mms(stat, xst):
                prx = pg.tile([128, 512], F32, tag="prx0")
                pnz = pg.tile([128, 512], F32, tag="pnz0")
                pr = prx[:, 0:256]
                px = prx[:, 256:512]
                pn = pnz[:, 0:256]
                pz = pnz[:, 256:512]
                for g in range(4):
                    nc.tensor.matmul(pr[32 * g:32 * g + B, :], xst,
                                     Wx0s[:, g, 0, :], start=True, stop=False,
                                     tile_position=(0, 32 * g))
                for ki, k in enumerate(KORD):
                    for g in range(4):
                        nc.tensor.matmul(pr[32 * g:32 * g + B, :], stat(k),
                                         Wh0s[:, k, g, 0, :], start=False,
                                         stop=(ki == KC - 1),
                                         tile_position=(0, 32 * g))
                for g in range(4):
                    nc.tensor.matmul(pn[32 * g:32 * g + B, :], onesb[:],
                                     bhn0s[:, g, :], start=True, stop=False,
                                     tile_position=(0, 32 * g))
                for ki, k in enumerate(KORD):
                    for g in range(4):
                        nc.tensor.matmul(pn[32 * g:32 * g + B, :], stat(k),
                                         Wh0s[:, k, g, 2, :], start=False,
                                         stop=(ki == KC - 1),
                                         tile_position=(0, 32 * g))
                for g in range(4):
                    nc.tensor.matmul(px[32 * g:32 * g + B, :], xst,
                                     Wx0s[:, g, 2, :], start=True, stop=True,
                                     tile_position=(0, 32 * g))
                for g in range(4):
                    nc.tensor.matmul(pz[32 * g:32 * g + B, :], xst,
                                     Wx0s[:, g, 1, :], start=True, stop=False,
                                     tile_position=(0, 32 * g))
                for ki, k in enumerate(KORD):
                    for g in range(4):
                        nc.tensor.matmul(pz[32 * g:32 * g + B, :], stat(k),
                                         Wh0s[:, k, g, 1, :], start=False,
                                         stop=(ki == KC - 1),
                                         tile_position=(0, 32 * g))
                return pr, pn, px, pz

            def l0_tail(pr, pn, px, pz):
                r_t = work.tile([128, 256], BF16, tag="r_t0")
                nc.scalar.activation(r_t[:], pr[:], AF.Sigmoid)
                t1 = work.tile([128, 256], BF16, tag="t10")
                nc.vector.scalar_tensor_tensor(t1[:], pn[:], 1.0, r_t[:],
                                               op0=ALU.mult, op1=ALU.mult)
                pre_n = work.tile([128, 256], BF16, tag="pre_n0")
                nc.vector.scalar_tensor_tensor(pre_n[:], t1[:], 0.0, px[:],
                                               op0=ALU.add, op1=ALU.add)
                n_t = work.tile([128, 256], BF16, tag="n_t0")
                nc.scalar.activation(n_t[:], pre_n[:], AF.Tanh)
                z_t = work.tile([128, 256], BF16, tag="z_t0")
                nc.scalar.activation(z_t[:], pz[:], AF.Sigmoid)
                d = work.tile([128, 256], BF16, tag="d0")
                nc.vector.tensor_sub(d[:], h0s[:], n_t[:])
                t2 = work.tile([128, 256], BF16, tag="t20")
                nc.vector.tensor_mul(t2[:], d[:], z_t[:])
                nc.vector.tensor_add(h0s[:], t2[:], n_t[:])

            def l0_tr(par, tau):
                pt = ptp.tile([128, 512], BF16, tag="ptx", name="ptx")[:, 0:256]
                for half in range(2):
                    nc.tensor.transpose(pt[:, 128 * half:128 * half + 128],
                                        h0s[:, 128 * half:128 * half + 128],
                                        idf[:])
                return pt

            # ---------------- layer 1 step ----------------
            def l1_r_mms(pr):
                for ki, k in enumerate(KORD):
                    for g in range(4):
                        nc.tensor.matmul(pr[32 * g:32 * g + B, :], hT2_chunk(k),
                                         Wh1s[:, k, g, 0, :], start=(ki == 0),
                                         stop=(ki == KC - 1),
                                         tile_position=(0, 32 * g))

            def l1_n_mms(pn):
                for g in range(4):
                    nc.tensor.matmul(pn[32 * g:32 * g + B, :], onesb[:],
                                     bhn1s[:, g, :], start=True, stop=False,
                                     tile_position=(0, 32 * g))
                for ki, k in enumerate(KORD):
                    for g in range(4):
                        nc.tensor.matmul(pn[32 * g:32 * g + B, :], hT2_chunk(k),
                                         Wh1s[:, k, g, 2, :], start=False,
                                         stop=(ki == KC - 1),
                                         tile_position=(0, 32 * g))

            def l1_z_mms(pz):
                for ki, k in enumerate(KORD):
                    for g in range(4):
                        nc.tensor.matmul(pz[32 * g:32 * g + B, :], hT2_chunk(k),
                                         Wh1s[:, k, g, 1, :], start=(ki == 0),
                                         stop=(ki == KC - 1),
                                         tile_position=(0, 32 * g))

            def l1_tail_a(pr, pn, xgpar, tau):
                xg = xg1c[xgpar]
                rpre = work.tile([128, 256], F32, tag="rpre1")
                nc.vector.scalar_tensor_tensor(rpre[:], pr[:], 0.0,
                                               xg[:, tau, 0, :],
                                               op0=ALU.add, op1=ALU.add)
                r_t = work.tile([128, 256], BF16, tag="r_t1")
                nc.scalar.activation(r_t[:], rpre[:], AF.Sigmoid)
                t1 = work.tile([128, 256], BF16, tag="t11")
                nc.vector.scalar_tensor_tensor(t1[:], pn[:], 1.0, r_t[:],
                                               op0=ALU.mult, op1=ALU.mult)
                pre_n = work.tile([128, 256], BF16, tag="pre_n1")
                nc.vector.scalar_tensor_tensor(pre_n[:], t1[:], 0.0,
                                               xg[:, tau, 2, :],
                                               op0=ALU.add, op1=ALU.add)
                n_t = work.tile([128, 256], BF16, tag="n_t1")
                nc.scalar.activation(n_t[:], pre_n[:], AF.Tanh)
                return n_t

            def l1_tail_b(pz, n_t, xgpar, tau):
                xg = xg1c[xgpar]
                zpre = work.tile([128, 256], F32, tag="zpre1")
                nc.vector.scalar_tensor_tensor(zpre[:], pz[:], 0.0,
                                               xg[:, tau, 1, :],
                                               op0=ALU.add, op1=ALU.add)
                z_t = work.tile([128, 256], BF16, tag="z_t1")
                nc.scalar.activation(z_t[:], zpre[:], AF.Sigmoid)
                d = work.tile([128, 256], BF16, tag="d1")
                nc.vector.tensor_sub(d[:], h2s[:], n_t[:])
                t2 = work.tile([128, 256], BF16, tag="t21")
                nc.vector.tensor_mul(t2[:], d[:], z_t[:])
                nc.vector.tensor_add(h2s[:], t2[:], n_t[:])

            def l1_tr():
                pt = ptp.tile([128, 512], BF16, tag="ptx", name="ptx")[:, 256:512]
                for half in range(2):
                    nc.tensor.transpose(pt[:, 128 * half:128 * half + 128],
                                        h2s[:, 128 * half:128 * half + 128],
                                        idf[:])
                return pt

            pend = [False]

            def flush_l1_tr():
                # Deferred L1 transpose+copy: emitted at the head of the NEXT
                # step-pair, when h2s is long since written, so the PE never
                # waits on the sigma-z/t2/h' chain.
                if not pend[0]:
                    return
                pend[0] = False
                pt1 = l1_tr()
                nc.vector.tensor_copy(hT2[:, 0, :], pt1[:, 0:128])
                nc.vector.tensor_copy(hT2[:, 1, :], pt1[:, 128:256])

            # ---------------- Wi1 GEMM for one chunk ----------------
            # M=32 stationary (8 batch x 4 steps) per col-strip, 4 strips
            # each streaming one g-block's 768 gate-columns (2 N-passes).
            PASSES = ((0, 512), (512, 768))

            def gemm_blocks(src_par, dst_par):
                buf = hT0c[src_par]

                def statg(k):
                    return buf[:, k % 2, 32 * (k // 2):32 * (k // 2) + B, 0:CH]

                emits = []
                for (c0, c1) in PASSES:
                    state = {}

                    def mk_bias(c0=c0, c1=c1, state=state):
                        def emit():
                            state["pgt"] = pgg_pool.tile([128, 512], F32,
                                                         tag="pgg", name="pgg")
                            for a in range(4):
                                nc.tensor.matmul(
                                    state["pgt"][32 * a:32 * a + 32, 0:c1 - c0],
                                    onesg[:], bi1s[:, a, c0:c1],
                                    start=True, stop=False,
                                    tile_position=(0, 32 * a))
                        return emit

                    def mk_k(k, c0=c0, c1=c1, state=state):
                        def emit():
                            for a in range(4):
                                nc.tensor.matmul(
                                    state["pgt"][32 * a:32 * a + 32, 0:c1 - c0],
                                    statg(k), Wi1s[:, k, a, c0:c1],
                                    start=False, stop=(k == KC - 1),
                                    tile_position=(0, 32 * a))
                        return emit

                    def mk_evac(c0=c0, c1=c1, state=state):
                        def emit():
                            nc.scalar.activation(tmp[dst_par][:, c0:c1],
                                                 state["pgt"][:, 0:c1 - c0],
                                                 AF.Copy)
                        return emit

                    emits.append(mk_bias())
                    for k in range(KC):
                        emits.append(mk_k(k))
                    emits.append(mk_evac())

                def dmas():
                    for g in range(4):
                        nc.sync.dma_start(
                            xg1c[dst_par][32 * g:32 * g + B, :, :, :],
                            tmp[dst_par][32 * g:32 * g + 32, :])
                emits.append(dmas)
                return emits

            def take(it, n):
                for _ in range(n):
                    b = next(it, None)
                    if b is not None:
                        b()

            # ---------------- slot emitters ----------------
            def emit_slot(l0_chunk_par, xst_of, gemm_emits, l1_on,
                          prefetch_of=None):
                """One slot: 8 interleaved L0/L1 steps + GEMM fillers.
                l0_chunk_par: parity of the L0 chunk being produced (or None).
                xst_of(u): stationary AP for L0 step u.
                gemm_emits: iterator of GEMM emit closures (or None).
                l1_on: (xg_parity, ) or None."""
                git = iter(gemm_emits) if gemm_emits is not None else iter(())
                for u in range(CH):
                    if l0_chunk_par is None:
                        flush_l1_tr()
                    else:
                        stat = hT0_stat(l0_chunk_par, u)
                        prpnpxpz = l0_mms(stat, xst_of(u))
                        flush_l1_tr()
                        l0_tail(*prpnpxpz)
                    if prefetch_of is not None:
                        prefetch_of(u)
                    pt0 = None
                    if l1_on is not None:
                        prz = pg.tile([128, 512], F32, tag="prz1", name="prz1")
                        pn1 = pg.tile([128, 256], F32, tag="pn1", name="pn1")
                        l1_r_mms(prz[:, 0:256])
                        l1_n_mms(pn1[:])
                        n_t1 = l1_tail_a(prz[:, 0:256], pn1[:], l1_on, u)
                        l1_z_mms(prz[:, 256:512])
                        l1_tail_b(prz[:, 256:512], n_t1, l1_on, u)
                    if l0_chunk_par is not None:
                        pt0 = l0_tr(l0_chunk_par, u)
                    take(git, 6)
                    if pt0 is not None:
                        nc.scalar.activation(hT0c[l0_chunk_par][:, 0, :, u],
                                             pt0[:, 0:128], AF.Copy)
                        nc.scalar.activation(hT0c[l0_chunk_par][:, 1, :, u],
                                             pt0[:, 128:256], AF.Copy)
                    if l1_on is not None:
                        pend[0] = True
                # drain remaining gemm blocks
                take(git, 40)

            # ---------------- prologue: chunks 0 and 1 ----------------
            def xst_static(c):
                return lambda u: xTs[:, CH * c + u, :]

            emit_slot(0, xst_static(0), None, None)

            xstE = [xstp.tile([128, B], BF16, tag=f"xstE{u}", name=f"xstE{u}") for u in range(CH)]
            xstO = [xstp.tile([128, B], BF16, tag=f"xstO{u}", name=f"xstO{u}") for u in range(CH)]

            def pro_prefetch(u):
                # preload xst tiles for main-loop iteration 0
                nc.scalar.activation(xstE[u][:], xTs[:, 2 * CH + u, :], AF.Copy)
                nc.scalar.activation(xstO[u][:], xTs[:, 3 * CH + u, :], AF.Copy)

            emit_slot(1, xst_static(1), gemm_blocks(0, 0), None,
                      prefetch_of=pro_prefetch)

            # ---------------- main loop: 15 iterations x 2 slots ----------
            pend[0] = True   # ensure the loop body traces the head flush
            n_iter = (NCH - 2) // 2 * repeat
            with tc.For_i(0, n_iter, 1, staggered_reset=True) as iv:
                for sl, xst_tiles in ((0, xstE), (1, xstO)):
                    base = 2 * CH + CH * sl

                    def xst_of(u, xst_tiles=xst_tiles):
                        return xst_tiles[u][:]

                    def prefetch(u, xst_tiles=xst_tiles, base=base):
                        if repeat == 1:
                            src = xTs[:, bass.ds(iv * 2 * CH + base + 2 * CH + u, 1), :].opt()
                        else:
                            src = xTs[:, base + u, :]
                        nc.scalar.activation(xst_tiles[u][:], src, AF.Copy)

                    l0_par = sl          # chunk 2i+2+sl has parity sl
                    gem = gemm_blocks(1 - sl, 1 - sl)   # chunk 2i+1+sl
                    l1_par = sl          # chunk 2i+sl
                    emit_slot(l0_par, xst_of, gem, l1_par, prefetch_of=prefetch)

            # ---------------- epilogue ----------------
            # GEMM(chunk 31, parity 1) + L1(chunk 30, parity 0)
            emit_slot(None, None, gemm_blocks(1, 1), 0)
            # L1(chunk 31, parity 1)
            emit_slot(None, None, None, 1)

            flush_l1_tr()

            # ---------------- final projection ----------------
            pfc = ptp.tile([B, 1], F32, tag="pfc")
            for k in range(KC):
                nc.tensor.matmul(pfc[:], hT2_chunk(k), fcws[:, k, :],
                                 start=(k == 0), stop=(k == KC - 1))
            ov = work.tile([B, 1], F32, tag="ov")
            nc.vector.tensor_copy(ov[:], pfc[:])
            nc.sync.dma_start(out_d.ap(), ov[:])

    nc.compile()
    return nc


def _prep_inputs(x, w_ih_l0, w_hh_l0, b_ih_l0, b_hh_l0,
                 w_ih_l1, w_hh_l1, b_ih_l1, b_hh_l1, fc_w, fc_b):
    bf = ml_dtypes.bfloat16
    f32 = np.float32
    x = np.asarray(x, f32)
    w_ih_l0 = np.asarray(w_ih_l0, f32); w_hh_l0 = np.asarray(w_hh_l0, f32)
    b_ih_l0 = np.asarray(b_ih_l0, f32); b_hh_l0 = np.asarray(b_hh_l0, f32)
    w_ih_l1 = np.asarray(w_ih_l1, f32); w_hh_l1 = np.asarray(w_hh_l1, f32)
    b_ih_l1 = np.asarray(b_ih_l1, f32); b_hh_l1 = np.asarray(b_hh_l1, f32)
    fc_w = np.asarray(fc_w, f32)

    def reorder_h(W):
        # W: [3072, 1024] -> [128(p), KC, 4(g), 3(gate), 256(c)]
        Wr = W.reshape(3, 4, 256, KC, 128)      # gate, g, c, k, p
        return np.ascontiguousarray(Wr.transpose(4, 3, 1, 0, 2))

    Wh0 = reorder_h(w_hh_l0).astype(bf)
    Wh1 = reorder_h(w_hh_l1).astype(bf)
    Wi1 = reorder_h(w_ih_l1)                     # [128, KC, 4, 3, 256]
    Wi1 = Wi1.reshape(128, KC, 4, 768).astype(bf)

    # bi1: row0 = input-side biases for l1 in (g, gate, c) flat order.
    bi = np.zeros((128, 4, 768), f32)
    bsum = b_ih_l1 + b_hh_l1
    bvec = np.empty((3, 4, 256), f32)
    bvec[0] = bsum[0:H].reshape(4, 256)
    bvec[1] = bsum[H:2 * H].reshape(4, 256)
    bvec[2] = b_ih_l1[2 * H:3 * H].reshape(4, 256)
    bi[0] = bvec.transpose(1, 0, 2).reshape(4, 768)
    bi1 = bi.astype(bf)

    # Wx0: [128, 4, 3, 256]; rows 0-63 w_ih_l0.T, row 64 biases (r,z: both; n: ih)
    Wx0 = np.zeros((128, 4, 3, 256), f32)
    wi0 = w_ih_l0.reshape(3, 4, 256, 64)         # gate, g, c, d
    Wx0[0:64] = wi0.transpose(3, 1, 0, 2)        # d, g, gate, c
    b0sum = b_ih_l0 + b_hh_l0
    Wx0[64, :, 0, :] = b0sum[0:H].reshape(4, 256)
    Wx0[64, :, 1, :] = b0sum[H:2 * H].reshape(4, 256)
    Wx0[64, :, 2, :] = b_ih_l0[2 * H:3 * H].reshape(4, 256)
    Wx0 = Wx0.astype(bf)

    bhn0 = np.zeros((128, 4, 256), f32)
    bhn0[0] = b_hh_l0[2 * H:3 * H].reshape(4, 256)
    bhn0 = bhn0.astype(bf)
    bhn1 = np.zeros((128, 4, 256), f32)
    bhn1[0] = b_hh_l1[2 * H:3 * H].reshape(4, 256)
    bhn1 = bhn1.astype(bf)

    fcw = np.ascontiguousarray(fc_w.reshape(KC, 128).T).reshape(128, KC, 1)
    fcw = fcw.astype(bf)
    ident = np.eye(128, dtype=f32).astype(bf)

    shared = dict(Wx0=Wx0, Wh0=Wh0, Wh1=Wh1, Wi1=Wi1, bi1=bi1,
                  bhn0=bhn0, bhn1=bhn1, ident=ident, fcw=fcw)
    in_maps = []
    for c in range(N_CORES):
        xs = x[c * B:(c + 1) * B]                 # [B, T, D]
        xTc = np.zeros((128, T + 2 * CH, B), f32)
        xTc[0:64, 0:T, :] = xs.transpose(2, 1, 0)
        xTc[64, 0:T, :] = 1.0
        m = dict(shared)
        m["xT"] = xTc.astype(bf)
        in_maps.append(m)
    return in_maps


def kernel(**inputs) -> np.ndarray:
    from concourse import bass_utils
    if "nc" not in _cache:
        _cache["nc"] = _build()
    nc = _cache["nc"]
    in_maps = _prep_inputs(**inputs)
    res = bass_utils.run_bass_kernel_spmd(nc, in_maps,
                                          core_ids=list(range(N_CORES)))
    out = np.concatenate([res.results[c]["out"] for c in range(N_CORES)], axis=0)
    return (out + np.asarray(inputs["fc_b"], np.float32)).astype(np.float32)
